# revision 3
# baseline (speedup 1.0000x reference)
"""Multi-head attention (B=4, N=2048, D=768, H=16) on 8 trn2 NeuronCores.

Sharding: core c = (batch b = c//2, head-group hg = c%2). Each core computes
attention for 8 heads of one batch element and the partial output projection
for those heads; the host sums the two partial projections per batch (the
tensor-parallel unshard) and adds the output bias.

Device kernel layout (per core) — matmul operands bf16, PSUM/softmax fp32:
  - All projections produce *transposed* activations: QT/KT [dim, seq] so the
    scores matmul S^T = K Q^T needs no transposes; softmax runs along q (free)
    with k on partitions; the row-sum for the softmax denominator is obtained
    by augmenting V with a ones column so the PV matmul emits it for free
    (rows 48/112 of the PSUM accumulator).
  - Heads are processed in pairs packed at partition offsets 0 and 64 so two
    K=48 (QK) / M=64 (PV) matmuls share the PE array via row/col tile groups.
  - exp() runs on the scalar engine straight out of PSUM in [128,1024] tiles,
    double buffered; the PV accumulator is bounced to SBUF in one copy so its
    PSUM banks free immediately for the next q-half.
  - The output projection accumulates across head pairs in SBUF on the vector
    engine; each e-chunk is DMA'd out right after its last accumulation.
  - The next pair's Q/K projection gens are interleaved into the attention
    loop to fill PE slack; tiny preamble matmuls warm each engine's semaphore
    clocks (walrus allows one wait per lowered instruction).
  - 1/sqrt(768) score scale is folded into WQ (and bQ) on the host.
"""

import math
import os

import numpy as np
import ml_dtypes

import concourse.bass as bass
import concourse.bacc as bacc
import concourse.tile as tile
from concourse import mybir
from concourse.bass_utils import run_bass_kernel_spmd
from contextlib import ExitStack

B, N, D, H, DH = 4, 2048, 768, 16, 48
P = 128
CC = D // P          # 6 contraction chunks of 128
KC = N // P          # 16 key chunks of 128
NPAIR = 4            # head pairs per core (8 heads)
QH = 1024            # q-half width (PSUM budget)
QB = 512             # matmul moving free dim (fp32 max)
F32 = mybir.dt.float32
BF16 = mybir.dt.bfloat16

_PROGRAM = None
LAST_RESULTS = None  # BassKernelResults of the most recent run (for test.py)


def _emit(ctx, tc, xt, wq, wk, wv, wp, bq, bk, bv, outt):
    nc = tc.nc
    Exp = mybir.ActivationFunctionType.Exp
    ADD = mybir.AluOpType.add
    MULT = mybir.AluOpType.mult

    consts = ctx.enter_context(tc.tile_pool(name="consts", bufs=1))
    qkvp = ctx.enter_context(tc.tile_pool(name="qkvp", bufs=2))
    vpool = ctx.enter_context(tc.tile_pool(name="vpool", bufs=1))
    ptp = ctx.enter_context(tc.tile_pool(name="ptp", bufs=4))
    ohp = ctx.enter_context(tc.tile_pool(name="ohp", bufs=2))
    rp = ctx.enter_context(tc.tile_pool(name="rp", bufs=1))
    otp = ctx.enter_context(tc.tile_pool(name="otp", bufs=3))
    sp = ctx.enter_context(tc.tile_pool(name="sp", bufs=1, space="PSUM"))
    ovp = ctx.enter_context(tc.tile_pool(name="ovp", bufs=1, space="PSUM"))
    pjp = ctx.enter_context(tc.tile_pool(name="pjp", bufs=2, space="PSUM"))

    # ---- constant loads ----
    wq_sb = consts.tile([P, NPAIR, CC, P], BF16)
    nc.sync.dma_start(out=wq_sb[:], in_=wq[:])
    wk_sb = consts.tile([P, NPAIR, CC, P], BF16)
    nc.sync.dma_start(out=wk_sb[:], in_=wk[:])
    bq_sb = consts.tile([P, NPAIR], F32)
    nc.sync.dma_start(out=bq_sb[:], in_=bq[:])
    bk_sb = consts.tile([P, NPAIR], F32)
    nc.sync.dma_start(out=bk_sb[:], in_=bk[:])
    xt_sb = consts.tile([P, CC, N], BF16)
    for c in range(CC):
        for h2 in range(2):
            nc.sync.dma_start(out=xt_sb[:, c, h2 * (N // 2):(h2 + 1) * (N // 2)],
                              in_=xt[:, c, h2 * (N // 2):(h2 + 1) * (N // 2)])
    wv_sb = consts.tile([P, CC, 8 * DH], BF16)
    nc.sync.dma_start(out=wv_sb[:], in_=wv[:])
    bv_sb = consts.tile([P, 8, DH], F32)
    nc.sync.dma_start(out=bv_sb[:], in_=bv[:])
    wp_sb = consts.tile([P, NPAIR, CC, P], BF16)
    nc.sync.dma_start(out=wp_sb[:], in_=wp[:])

    # ---- engine-clock warm-up ----
    # A self-loading fp32 matmul carries at most ONE semaphore wait in its
    # lowered form, so no real matmul may be the first observer of two new
    # semaphores.  Touch every DMA-loaded operand with a tiny dummy matmul
    # (PE) / copy (DVE) so each engine observes every DMA queue's semaphore
    # before real work begins.
    junk = pjp.tile([P, QB], F32, name="pj")
    for wi, ap in enumerate((
        wq_sb[0:1, 0, 0, 0:1], wk_sb[0:1, 0, 0, 0:1], wv_sb[0:1, 0, 0:1],
        wp_sb[0:1, 0, 0, 0:1],
        xt_sb[0:1, 0, 0:1], xt_sb[0:1, 1, 0:1], xt_sb[0:1, 2, 0:1],
        xt_sb[0:1, 3, 0:1], xt_sb[0:1, 4, 0:1], xt_sb[0:1, 5, 0:1],
    )):
        nc.tensor.matmul(junk[0:1, wi:wi + 1], lhsT=ap, rhs=ap,
                         start=True, stop=True)
    scr = otp.tile([P, QB], F32, name="ot")
    nc.vector.tensor_copy(scr[0:1, 0:1], bq_sb[0:1, 0:1])
    nc.vector.tensor_copy(scr[0:1, 1:2], bk_sb[0:1, 0:1])
    nc.vector.tensor_copy(scr[0:1, 2:3], bv_sb[0:1, 0, 0:1])

    # ---- Q/K projections (pair-packed transposed layout [128, 2048]) ----
    def project_qk(p):
        outs = []
        for w_sb, b_sb, nm in ((wq_sb, bq_sb, "qt"), (wk_sb, bk_sb, "kt")):
            dst = qkvp.tile([P, N], BF16, name=nm)
            for qb in range(N // QB):
                ps = pjp.tile([P, QB], F32, name="pj")
                for c in range(CC):
                    nc.tensor.matmul(
                        ps[:],
                        lhsT=w_sb[:, p, c, :],
                        rhs=xt_sb[:, c, qb * QB:(qb + 1) * QB],
                        start=(c == 0),
                        stop=(c == CC - 1),
                    )
                nc.vector.tensor_scalar_add(
                    dst[:, qb * QB:(qb + 1) * QB], ps[:], b_sb[:, p:p + 1]
                )
            outs.append(dst)
        return outs

    # ---- V projection: [k-part, k-chunk, head, 64]: 48 dims | ones | zeros.
    # The ones column makes the PV matmul emit softmax row-sums at psum row
    # 48/112 for free; padding to 64 makes PV write full 32-row quadrants so
    # no PSUM row is ever read uninitialized.
    v_sb = vpool.tile([P, KC, 8, 64], BF16)
    nc.vector.memset(v_sb[:, :, :, DH:DH + 1], 1.0)
    nc.vector.memset(v_sb[:, :, :, DH + 1:64], 0.0)

    def emit_v_gen(s):
        ps = pjp.tile([P, QB], F32, name="pj")
        for c in range(CC):
            nc.tensor.matmul(
                ps[:, 0:8 * DH],
                lhsT=xt_sb[:, c, s * P:(s + 1) * P],
                rhs=wv_sb[:, c, :],
                start=(c == 0),
                stop=(c == CC - 1),
            )
        nc.vector.scalar_tensor_tensor(
            out=v_sb[:, s, :, 0:DH],
            in0=ps[:, 0:8 * DH].rearrange("p (h d) -> p h d", h=8),
            scalar=1.0,
            in1=bv_sb[:],
            op0=MULT,
            op1=ADD,
        )

    qt_cur, kt_cur = project_qk(0)
    # V chunks 0-3 upfront; chunks 4-15 interleave into pair 0's attention
    # loop (the PV matmul for chunk i only needs V chunk i, and the pt pool
    # lets the PV stream lag the exp stream) so the first exp starts ~25us
    # earlier instead of waiting for the whole V projection.
    for s in range(4):
        emit_v_gen(s)

    # SBUF accumulator for the output projection (summed over pairs on DVE;
    # one DMA store at the end instead of per-pair DMA-accumulate round trips)
    oaccp = ctx.enter_context(tc.tile_pool(name="oaccp", bufs=1))
    out_acc = oaccp.tile([P, CC, N], F32)

    def emit_qk_gen(w_sb, b_sb, dst, pr, qb):
        ps = pjp.tile([P, QB], F32, name="pj")
        for c in range(CC):
            nc.tensor.matmul(
                ps[:],
                lhsT=w_sb[:, pr, c, :],
                rhs=xt_sb[:, c, qb * QB:(qb + 1) * QB],
                start=(c == 0),
                stop=(c == CC - 1),
            )
        nc.vector.tensor_scalar_add(
            dst[:, qb * QB:(qb + 1) * QB], ps[:], b_sb[:, pr:pr + 1]
        )

    # ---- attention + incremental output projection ----
    for p in range(NPAIR):
        # absorb the DVE tick of this pair's fresh QT/KT evictions into PE's
        # clock so the first scores matmul needs only its PSUM-slot wait
        junk2 = pjp.tile([P, QB], F32, name="pj")
        nc.tensor.matmul(junk2[0:1, 0:1], lhsT=qt_cur[0:1, 0:1],
                         rhs=qt_cur[0:1, 0:1], start=True, stop=True)
        nc.tensor.matmul(junk2[0:1, 1:2], lhsT=kt_cur[0:1, 0:1],
                         rhs=kt_cur[0:1, 0:1], start=True, stop=True)
        # next pair's Q/K projection gens, interleaved into this pair's
        # attention loop so they fill PE slack instead of serializing at the
        # pair boundary
        pending = []
        if p == 0:
            pending += [(lambda s=s: emit_v_gen(s)) for s in range(4, KC)]
        if p + 1 < NPAIR:
            qt_nxt = qkvp.tile([P, N], BF16, name="qt")
            kt_nxt = qkvp.tile([P, N], BF16, name="kt")
            pending += [
                (lambda qb=qb: emit_qk_gen(wq_sb, bq_sb, qt_nxt, p + 1, qb))
                for qb in range(4)
            ] + [
                (lambda qb=qb: emit_qk_gen(wk_sb, bk_sb, kt_nxt, p + 1, qb))
                for qb in range(4)
            ]
        for qh in range(N // QH):
            q0 = qh * QH
            ov = ovp.tile([P, QH], F32, name="ov")
            for i in range(KC):
                # scores quad, j-major: the two heads' matmuls land on
                # disjoint PE row groups (rows 0-63 / 64-127) and issue
                # back-to-back, so they stream concurrently (~2x).
                s_tiles = (sp.tile([P, QH], F32, name="s0"),
                           sp.tile([P, QH], F32, name="s1"))
                for j in range(QH // QB):
                    for hh in range(2):
                        row0 = 64 * hh
                        nc.tensor.matmul(
                            s_tiles[hh][:, j * QB:(j + 1) * QB],
                            lhsT=kt_cur[row0:row0 + DH, i * P:(i + 1) * P],
                            rhs=qt_cur[row0:row0 + DH, q0 + j * QB:q0 + (j + 1) * QB],
                            start=True,
                            stop=True,
                            tile_position=(row0, 0),
                        )
                pt_tiles = (ptp.tile([P, QH], BF16, name="pt0"),
                            ptp.tile([P, QH], BF16, name="pt1"))
                for hh in range(2):
                    nc.scalar.activation(pt_tiles[hh][:], s_tiles[hh][:], Exp)
                # PV quad, j-major: the two heads' matmuls land on disjoint
                # PE col groups (cols 0-63 / 64-127) -> concurrent streams.
                for j in range(QH // QB):
                    for hh in range(2):
                        row0 = 64 * hh
                        nc.tensor.matmul(
                            ov[row0:row0 + 64, j * QB:(j + 1) * QB],
                            lhsT=v_sb[:, i, 2 * p + hh, :],
                            rhs=pt_tiles[hh][:, j * QB:(j + 1) * QB],
                            start=(i == 0),
                            stop=(i == KC - 1),
                            tile_position=(0, row0),
                            skip_group_check=True,
                        )
                fire = (qh == 0) if p == 0 else (i % 4 == 1)
                if (fire or i % 4 == 1) and pending:
                    pending.pop(0)()
            # Bounce the PV accumulator to SBUF in one copy so the PSUM tile
            # frees immediately for the next q-half's PV matmuls.
            ovs = rp.tile([P, QH], F32, name="ovs")
            nc.vector.tensor_copy(ovs[:], ov[:])
            # softmax normalization: divide by the ones-column row-sums
            # (rows 48 / 112).  Those rows are partition-misaligned for
            # compute ops, so stream_shuffle (a per-32-lane crossbar) first
            # broadcasts them across the aligned quadrants; one reciprocal
            # and one multiply per head then normalizes + evicts.
            oh = ohp.tile([P, QH], BF16, name="oh")
            rec = rp.tile([P, QH], F32, name="rec")
            bc = rp.tile([P, QH], F32, name="bc")
            m16 = [16] * 32
            nc.vector.stream_shuffle(bc[0:32, :], ovs[32:64, :], m16)
            nc.vector.stream_shuffle(bc[32:64, :], ovs[32:64, :], m16)
            nc.vector.stream_shuffle(bc[64:96, :], ovs[96:P, :], m16)
            nc.vector.stream_shuffle(bc[96:P, :], ovs[96:P, :], m16)
            # rowsums are ~2048 (safely inside approx range); 18-bit recip is
            # far below the bf16 noise floor and ~5x cheaper than exact
            nc.vector.reciprocal_approx_fast(rec[0:P, :], bc[0:P, :])
            nc.vector.memset(oh[32:64, :], 0.0)
            nc.vector.memset(oh[96:P, :], 0.0)
            nc.vector.tensor_mul(oh[0:DH, :], ovs[0:DH, :], rec[0:DH, :])
            nc.vector.tensor_mul(oh[64:64 + DH, :], ovs[64:64 + DH, :],
                                 rec[64:64 + DH, :])
            # partial output projection, accumulated in SBUF across pairs
            for e in range(CC):
                for qs in range(QH // QB):
                    pp = pjp.tile([P, QB], F32, name="pj")
                    nc.tensor.matmul(
                        pp[:],
                        lhsT=wp_sb[:, p, e, :],
                        rhs=oh[:, qs * QB:(qs + 1) * QB],
                        start=True,
                        stop=True,
                    )
                    dst = out_acc[:, e, q0 + qs * QB:q0 + (qs + 1) * QB]
                    if p == 0:
                        nc.vector.tensor_copy(dst, pp[:])
                    else:
                        nc.vector.scalar_tensor_tensor(
                            out=dst, in0=pp[:], scalar=1.0, in1=dst,
                            op0=MULT, op1=ADD,
                        )
                if p == NPAIR - 1 and qh == 1:
                    # last accumulation for this e-chunk: store it now so the
                    # output DMA overlaps the remaining projection work
                    nc.sync.dma_start(out=outt[e], in_=out_acc[:, e, :])
        while pending:
            pending.pop(0)()
        if p + 1 < NPAIR:
            qt_cur, kt_cur = qt_nxt, kt_nxt


def _build_program():
    # Bacc (not plain Bass): its compile pipeline legalizes semaphore waits
    # (move_matmul_waits_to_ldweights / generate_event_semaphores) for the
    # 1-wait-per-instruction TRN2 constraint walrus enforces.
    nc = bacc.Bacc(None)
    xt = nc.dram_tensor("xt", [P, CC, N], BF16, kind="ExternalInput")
    wq = nc.dram_tensor("wq", [P, NPAIR, CC, P], BF16, kind="ExternalInput")
    wk = nc.dram_tensor("wk", [P, NPAIR, CC, P], BF16, kind="ExternalInput")
    wv = nc.dram_tensor("wv", [P, CC, 8 * DH], BF16, kind="ExternalInput")
    wp = nc.dram_tensor("wp", [P, NPAIR, CC, P], BF16, kind="ExternalInput")
    bq = nc.dram_tensor("bq", [P, NPAIR], F32, kind="ExternalInput")
    bk = nc.dram_tensor("bk", [P, NPAIR], F32, kind="ExternalInput")
    bv = nc.dram_tensor("bv", [P, 8, DH], F32, kind="ExternalInput")
    outt = nc.dram_tensor("outt", [CC, P, N], F32, kind="ExternalOutput")
    with tile.TileContext(nc) as tc:
        with ExitStack() as ctx:
            _emit(ctx, tc, xt, wq, wk, wv, wp, bq, bk, bv, outt)
    nc.finalize()
    return nc


def _get_program():
    global _PROGRAM
    if _PROGRAM is None:
        _PROGRAM = _build_program()
    return _PROGRAM


def _bf16(a):
    return np.ascontiguousarray(a.astype(ml_dtypes.bfloat16))


def _pairize_cols(W, hg, scale=1.0):
    """[768, 768] -> [768, 512]: pair p gets head hg*8+2p at cols 0:48 and
    head hg*8+2p+1 at cols 64:112 of its 128-col block; the rest zeros."""
    Wp = np.zeros((D, 512), np.float32)
    for p in range(NPAIR):
        ha = (hg * 8 + 2 * p) * DH
        hb = (hg * 8 + 2 * p + 1) * DH
        Wp[:, p * P:p * P + DH] = W[:, ha:ha + DH]
        Wp[:, p * P + 64:p * P + 64 + DH] = W[:, hb:hb + DH]
    if scale != 1.0:
        Wp *= scale
    return Wp


def _pairize_bias(b, hg, scale=1.0):
    bp = np.zeros((P, NPAIR), np.float32)
    for p in range(NPAIR):
        ha = (hg * 8 + 2 * p) * DH
        hb = (hg * 8 + 2 * p + 1) * DH
        bp[0:DH, p] = b[ha:ha + DH]
        bp[64:64 + DH, p] = b[hb:hb + DH]
    if scale != 1.0:
        bp *= scale
    return bp


def _prep_inputs(inputs):
    x = np.asarray(inputs["x"], np.float32)
    WQ = np.asarray(inputs["WQ"], np.float32)
    WK = np.asarray(inputs["WK"], np.float32)
    WV = np.asarray(inputs["WV"], np.float32)
    WP = np.asarray(inputs["WP"], np.float32)
    bQ = np.asarray(inputs["bQ"], np.float32)
    bK = np.asarray(inputs["bK"], np.float32)
    bV = np.asarray(inputs["bV"], np.float32)
    scale = 1.0 / math.sqrt(D)

    per_hg = {}
    for hg in range(2):
        wq_d = _bf16(_pairize_cols(WQ, hg, scale).reshape(CC, P, NPAIR, P).transpose(1, 2, 0, 3))
        wk_d = _bf16(_pairize_cols(WK, hg).reshape(CC, P, NPAIR, P).transpose(1, 2, 0, 3))
        wv_d = _bf16(WV[:, hg * 384:(hg + 1) * 384].reshape(CC, P, 384).transpose(1, 0, 2))
        WPpad = np.zeros((NPAIR, P, D), np.float32)
        for p in range(NPAIR):
            ha = (hg * 8 + 2 * p) * DH
            hb = (hg * 8 + 2 * p + 1) * DH
            WPpad[p, 0:DH] = WP[ha:ha + DH, :]
            WPpad[p, 64:64 + DH] = WP[hb:hb + DH, :]
        wp_d = _bf16(WPpad.reshape(NPAIR, P, CC, P).transpose(1, 0, 2, 3))
        bq_d = _pairize_bias(bQ, hg, scale)
        bk_d = _pairize_bias(bK, hg)
        bv_d = np.ascontiguousarray(
            np.broadcast_to(bV[hg * 384:(hg + 1) * 384].reshape(8, DH), (P, 8, DH))
        )
        per_hg[hg] = dict(wq=wq_d, wk=wk_d, wv=wv_d, wp=wp_d, bq=bq_d, bk=bk_d, bv=bv_d)

    in_maps = []
    for c in range(8):
        b, hg = c // 2, c % 2
        xt_d = _bf16(x[b].T.reshape(CC, P, N).transpose(1, 0, 2))
        m = dict(per_hg[hg])
        m["xt"] = xt_d
        in_maps.append(m)
    return in_maps


def kernel(**inputs):
    global LAST_RESULTS
    bP = np.asarray(inputs["bP"], np.float32)
    nc = _get_program()
    in_maps = _prep_inputs(inputs)
    trace = bool(os.environ.get("BASS_KERNEL_TRACE"))
    tmpdir = os.environ.get("BASS_KERNEL_TMPDIR") or None
    res = run_bass_kernel_spmd(nc, in_maps, list(range(8)), trace=trace, tmpdir=tmpdir)
    LAST_RESULTS = res
    out = np.empty((B, N, D), np.float32)
    for b in range(B):
        t = res.results[2 * b]["outt"].reshape(D, N) + \
            res.results[2 * b + 1]["outt"].reshape(D, N)
        out[b] = t.T + bP
    return out



# revision 4
# speedup vs baseline: 1.2165x; 1.2165x over previous
"""Multi-head attention (B=4, N=2048, D=768, H=16) on 8 trn2 NeuronCores.

Sharding: core c = (batch b = c//2, head-group hg = c%2). Each core computes
attention for 8 heads of one batch element and the partial output projection
for those heads; the host sums the two partial projections per batch (the
tensor-parallel unshard) and adds the output bias.

Device kernel layout (per core) — matmul operands bf16, PSUM/softmax fp32:
  - All projections produce *transposed* activations: QT/KT [dim, seq] so the
    scores matmul S^T = K Q^T needs no transposes; softmax runs along q (free)
    with k on partitions; the row-sum for the softmax denominator is obtained
    by augmenting V with a ones column so the PV matmul emits it for free
    (rows 48/112 of the PSUM accumulator).
  - Heads are processed in pairs packed at partition offsets 0 and 64 so two
    K=48 (QK) / M=64 (PV) matmuls share the PE array via row/col tile groups.
  - exp() runs on the scalar engine straight out of PSUM in [128,1024] tiles,
    double buffered; the PV accumulator is bounced to SBUF in one copy so its
    PSUM banks free immediately for the next q-half.
  - The output projection accumulates across head pairs in SBUF on the vector
    engine; each e-chunk is DMA'd out right after its last accumulation.
  - The next pair's Q/K projection gens are interleaved into the attention
    loop to fill PE slack; tiny preamble matmuls warm each engine's semaphore
    clocks (walrus allows one wait per lowered instruction).
  - 1/sqrt(768) score scale is folded into WQ (and bQ) on the host.
"""

import math
import os

import numpy as np
import ml_dtypes

import concourse.bass as bass
import concourse.bacc as bacc
import concourse.tile as tile
from concourse import mybir
from concourse.bass_utils import run_bass_kernel_spmd
from contextlib import ExitStack

B, N, D, H, DH = 4, 2048, 768, 16, 48
P = 128
CC = D // P          # 6 contraction chunks of 128
KC = N // P          # 16 key chunks of 128
NPAIR = 4            # head pairs per core (8 heads)
QH = 1024            # q-half width (PSUM budget)
QB = 512             # matmul moving free dim (fp32 max)
F32 = mybir.dt.float32
BF16 = mybir.dt.bfloat16

_PROGRAM = None
LAST_RESULTS = None  # BassKernelResults of the most recent run (for test.py)


def _emit(ctx, tc, xt, wq, wk, wv, wp, bq, bk, bv, outt):
    nc = tc.nc
    Exp = mybir.ActivationFunctionType.Exp
    ADD = mybir.AluOpType.add
    MULT = mybir.AluOpType.mult

    consts = ctx.enter_context(tc.tile_pool(name="consts", bufs=1))
    qkvp = ctx.enter_context(tc.tile_pool(name="qkvp", bufs=2))
    vpool = ctx.enter_context(tc.tile_pool(name="vpool", bufs=1))
    ptp = ctx.enter_context(tc.tile_pool(name="ptp", bufs=4))
    ohp = ctx.enter_context(tc.tile_pool(name="ohp", bufs=2))
    rp = ctx.enter_context(tc.tile_pool(name="rp", bufs=1))
    otp = ctx.enter_context(tc.tile_pool(name="otp", bufs=3))
    sp = ctx.enter_context(tc.tile_pool(name="sp", bufs=2, space="PSUM"))
    ovp = ctx.enter_context(tc.tile_pool(name="ovp", bufs=1, space="PSUM"))
    pjp = ctx.enter_context(tc.tile_pool(name="pjp", bufs=2, space="PSUM"))

    # ---- constant loads ----
    wq_sb = consts.tile([P, NPAIR, CC, P], BF16)
    nc.sync.dma_start(out=wq_sb[:], in_=wq[:])
    wk_sb = consts.tile([P, NPAIR, CC, P], BF16)
    nc.sync.dma_start(out=wk_sb[:], in_=wk[:])
    bq_sb = consts.tile([P, NPAIR], F32)
    nc.sync.dma_start(out=bq_sb[:], in_=bq[:])
    bk_sb = consts.tile([P, NPAIR], F32)
    nc.sync.dma_start(out=bk_sb[:], in_=bk[:])
    xt_sb = consts.tile([P, CC, N], BF16)
    for c in range(CC):
        for h2 in range(2):
            nc.sync.dma_start(out=xt_sb[:, c, h2 * (N // 2):(h2 + 1) * (N // 2)],
                              in_=xt[:, c, h2 * (N // 2):(h2 + 1) * (N // 2)])
    wv_sb = consts.tile([P, CC, 8 * DH], BF16)
    nc.sync.dma_start(out=wv_sb[:], in_=wv[:])
    bv_sb = consts.tile([P, 8, DH], F32)
    nc.sync.dma_start(out=bv_sb[:], in_=bv[:])
    wp_sb = consts.tile([P, NPAIR, CC, P], BF16)
    nc.sync.dma_start(out=wp_sb[:], in_=wp[:])

    # ---- engine-clock warm-up ----
    # A self-loading fp32 matmul carries at most ONE semaphore wait in its
    # lowered form, so no real matmul may be the first observer of two new
    # semaphores.  Touch every DMA-loaded operand with a tiny dummy matmul
    # (PE) / copy (DVE) so each engine observes every DMA queue's semaphore
    # before real work begins.
    junk = pjp.tile([P, QB], F32, name="pj")
    for wi, ap in enumerate((
        wq_sb[0:1, 0, 0, 0:1], wk_sb[0:1, 0, 0, 0:1], wv_sb[0:1, 0, 0:1],
        wp_sb[0:1, 0, 0, 0:1],
        xt_sb[0:1, 0, 0:1], xt_sb[0:1, 1, 0:1], xt_sb[0:1, 2, 0:1],
        xt_sb[0:1, 3, 0:1], xt_sb[0:1, 4, 0:1], xt_sb[0:1, 5, 0:1],
    )):
        nc.tensor.matmul(junk[0:1, wi:wi + 1], lhsT=ap, rhs=ap,
                         start=True, stop=True)
    scr = otp.tile([P, QB], F32, name="ot")
    nc.vector.tensor_copy(scr[0:1, 0:1], bq_sb[0:1, 0:1])
    nc.vector.tensor_copy(scr[0:1, 1:2], bk_sb[0:1, 0:1])
    nc.vector.tensor_copy(scr[0:1, 2:3], bv_sb[0:1, 0, 0:1])

    # ---- Q/K projections (pair-packed transposed layout [128, 2048]) ----
    def project_qk(p):
        outs = []
        for w_sb, b_sb, nm in ((wq_sb, bq_sb, "qt"), (wk_sb, bk_sb, "kt")):
            dst = qkvp.tile([P, N], BF16, name=nm)
            for qb in range(N // QB):
                ps = pjp.tile([P, QB], F32, name="pj")
                for c in range(CC):
                    nc.tensor.matmul(
                        ps[:],
                        lhsT=w_sb[:, p, c, :],
                        rhs=xt_sb[:, c, qb * QB:(qb + 1) * QB],
                        start=(c == 0),
                        stop=(c == CC - 1),
                    )
                nc.vector.tensor_scalar_add(
                    dst[:, qb * QB:(qb + 1) * QB], ps[:], b_sb[:, p:p + 1]
                )
            outs.append(dst)
        return outs

    # ---- V projection: [k-part, k-chunk, head, 64]: 48 dims | ones | zeros.
    # The ones column makes the PV matmul emit softmax row-sums at psum row
    # 48/112 for free; padding to 64 makes PV write full 32-row quadrants so
    # no PSUM row is ever read uninitialized.
    v_sb = vpool.tile([P, KC, 8, 64], BF16)
    nc.vector.memset(v_sb[:, :, :, DH:DH + 1], 1.0)
    nc.vector.memset(v_sb[:, :, :, DH + 1:64], 0.0)

    def emit_v_gen(s):
        ps = pjp.tile([P, QB], F32, name="pj")
        for c in range(CC):
            nc.tensor.matmul(
                ps[:, 0:8 * DH],
                lhsT=xt_sb[:, c, s * P:(s + 1) * P],
                rhs=wv_sb[:, c, :],
                start=(c == 0),
                stop=(c == CC - 1),
            )
        nc.vector.scalar_tensor_tensor(
            out=v_sb[:, s, :, 0:DH],
            in0=ps[:, 0:8 * DH].rearrange("p (h d) -> p h d", h=8),
            scalar=1.0,
            in1=bv_sb[:],
            op0=MULT,
            op1=ADD,
        )

    qt_cur, kt_cur = project_qk(0)
    # V chunks 0-3 upfront; chunks 4-15 interleave into pair 0's attention
    # loop (the PV matmul for chunk i only needs V chunk i, and the pt pool
    # lets the PV stream lag the exp stream) so the first exp starts ~25us
    # earlier instead of waiting for the whole V projection.
    for s in range(4):
        emit_v_gen(s)

    # SBUF accumulator for the output projection (summed over pairs on DVE;
    # one DMA store at the end instead of per-pair DMA-accumulate round trips)
    oaccp = ctx.enter_context(tc.tile_pool(name="oaccp", bufs=1))
    out_acc = oaccp.tile([P, CC, N], F32)

    def emit_qk_gen(w_sb, b_sb, dst, pr, qb):
        ps = pjp.tile([P, QB], F32, name="pj")
        for c in range(CC):
            nc.tensor.matmul(
                ps[:],
                lhsT=w_sb[:, pr, c, :],
                rhs=xt_sb[:, c, qb * QB:(qb + 1) * QB],
                start=(c == 0),
                stop=(c == CC - 1),
            )
        nc.vector.tensor_scalar_add(
            dst[:, qb * QB:(qb + 1) * QB], ps[:], b_sb[:, pr:pr + 1]
        )

    # ---- attention + incremental output projection ----
    for p in range(NPAIR):
        # absorb the DVE tick of this pair's fresh QT/KT evictions into PE's
        # clock so the first scores matmul needs only its PSUM-slot wait
        junk2 = pjp.tile([P, QB], F32, name="pj")
        nc.tensor.matmul(junk2[0:1, 0:1], lhsT=qt_cur[0:1, 0:1],
                         rhs=qt_cur[0:1, 0:1], start=True, stop=True)
        nc.tensor.matmul(junk2[0:1, 1:2], lhsT=kt_cur[0:1, 0:1],
                         rhs=kt_cur[0:1, 0:1], start=True, stop=True)
        # next pair's Q/K projection gens, interleaved into this pair's
        # attention loop so they fill PE slack instead of serializing at the
        # pair boundary
        pending = []
        if p == 0:
            pending += [(lambda s=s: emit_v_gen(s)) for s in range(4, KC)]
        if p + 1 < NPAIR:
            qt_nxt = qkvp.tile([P, N], BF16, name="qt")
            kt_nxt = qkvp.tile([P, N], BF16, name="kt")
            pending += [
                (lambda qb=qb: emit_qk_gen(wq_sb, bq_sb, qt_nxt, p + 1, qb))
                for qb in range(4)
            ] + [
                (lambda qb=qb: emit_qk_gen(wk_sb, bk_sb, kt_nxt, p + 1, qb))
                for qb in range(4)
            ]
        for qh in range(N // QH):
            q0 = qh * QH
            ov = ovp.tile([P, QH], F32, name="ov")
            for i in range(KC):
                for hh in range(2):
                    row0 = 64 * hh
                    s_ps = sp.tile([P, QH], F32, name="s")
                    for j in range(QH // QB):
                        nc.tensor.matmul(
                            s_ps[:, j * QB:(j + 1) * QB],
                            lhsT=kt_cur[row0:row0 + DH, i * P:(i + 1) * P],
                            rhs=qt_cur[row0:row0 + DH, q0 + j * QB:q0 + (j + 1) * QB],
                            start=True,
                            stop=True,
                            tile_position=(row0, 0),
                        )
                    pt = ptp.tile([P, QH], BF16, name="pt")
                    nc.scalar.activation(pt[:], s_ps[:], Exp)
                    for j in range(QH // QB):
                        nc.tensor.matmul(
                            ov[row0:row0 + 64, j * QB:(j + 1) * QB],
                            lhsT=v_sb[:, i, 2 * p + hh, :],
                            rhs=pt[:, j * QB:(j + 1) * QB],
                            start=(i == 0),
                            stop=(i == KC - 1),
                            tile_position=(0, row0),
                            skip_group_check=True,
                        )
                fire = (qh == 0) if p == 0 else (i % 4 == 1)
                if (fire or i % 4 == 1) and pending:
                    pending.pop(0)()
            # Bounce the PV accumulator to SBUF in one copy so the PSUM tile
            # frees immediately for the next q-half's PV matmuls.
            ovs = rp.tile([P, QH], F32, name="ovs")
            nc.vector.tensor_copy(ovs[:], ov[:])
            # softmax normalization: divide by the ones-column row-sums
            # (rows 48 / 112).  Those rows are partition-misaligned for
            # compute ops, so stream_shuffle (a per-32-lane crossbar) first
            # broadcasts them across the aligned quadrants; one reciprocal
            # and one multiply per head then normalizes + evicts.
            oh = ohp.tile([P, QH], BF16, name="oh")
            rec = rp.tile([P, QH], F32, name="rec")
            bc = rp.tile([P, QH], F32, name="bc")
            m16 = [16] * 32
            nc.vector.stream_shuffle(bc[0:32, :], ovs[32:64, :], m16)
            nc.vector.stream_shuffle(bc[32:64, :], ovs[32:64, :], m16)
            nc.vector.stream_shuffle(bc[64:96, :], ovs[96:P, :], m16)
            nc.vector.stream_shuffle(bc[96:P, :], ovs[96:P, :], m16)
            # rowsums are ~2048 (safely inside approx range); 18-bit recip is
            # far below the bf16 noise floor and ~5x cheaper than exact
            nc.vector.reciprocal_approx_fast(rec[0:P, :], bc[0:P, :])
            nc.vector.memset(oh[32:64, :], 0.0)
            nc.vector.memset(oh[96:P, :], 0.0)
            nc.vector.tensor_mul(oh[0:DH, :], ovs[0:DH, :], rec[0:DH, :])
            nc.vector.tensor_mul(oh[64:64 + DH, :], ovs[64:64 + DH, :],
                                 rec[64:64 + DH, :])
            # partial output projection, accumulated in SBUF across pairs
            for e in range(CC):
                for qs in range(QH // QB):
                    pp = pjp.tile([P, QB], F32, name="pj")
                    nc.tensor.matmul(
                        pp[:],
                        lhsT=wp_sb[:, p, e, :],
                        rhs=oh[:, qs * QB:(qs + 1) * QB],
                        start=True,
                        stop=True,
                    )
                    dst = out_acc[:, e, q0 + qs * QB:q0 + (qs + 1) * QB]
                    if p == 0:
                        nc.vector.tensor_copy(dst, pp[:])
                    else:
                        nc.vector.scalar_tensor_tensor(
                            out=dst, in0=pp[:], scalar=1.0, in1=dst,
                            op0=MULT, op1=ADD,
                        )
                if p == NPAIR - 1 and qh == 1:
                    # last accumulation for this e-chunk: store it now so the
                    # output DMA overlaps the remaining projection work
                    nc.sync.dma_start(out=outt[e], in_=out_acc[:, e, :])
        while pending:
            pending.pop(0)()
        if p + 1 < NPAIR:
            qt_cur, kt_cur = qt_nxt, kt_nxt


def _build_program():
    # Bacc (not plain Bass): its compile pipeline legalizes semaphore waits
    # (move_matmul_waits_to_ldweights / generate_event_semaphores) for the
    # 1-wait-per-instruction TRN2 constraint walrus enforces.
    nc = bacc.Bacc(None)
    xt = nc.dram_tensor("xt", [P, CC, N], BF16, kind="ExternalInput")
    wq = nc.dram_tensor("wq", [P, NPAIR, CC, P], BF16, kind="ExternalInput")
    wk = nc.dram_tensor("wk", [P, NPAIR, CC, P], BF16, kind="ExternalInput")
    wv = nc.dram_tensor("wv", [P, CC, 8 * DH], BF16, kind="ExternalInput")
    wp = nc.dram_tensor("wp", [P, NPAIR, CC, P], BF16, kind="ExternalInput")
    bq = nc.dram_tensor("bq", [P, NPAIR], F32, kind="ExternalInput")
    bk = nc.dram_tensor("bk", [P, NPAIR], F32, kind="ExternalInput")
    bv = nc.dram_tensor("bv", [P, 8, DH], F32, kind="ExternalInput")
    outt = nc.dram_tensor("outt", [CC, P, N], F32, kind="ExternalOutput")
    with tile.TileContext(nc) as tc:
        with ExitStack() as ctx:
            _emit(ctx, tc, xt, wq, wk, wv, wp, bq, bk, bv, outt)
    nc.finalize()
    return nc


def _get_program():
    global _PROGRAM
    if _PROGRAM is None:
        _PROGRAM = _build_program()
    return _PROGRAM


def _bf16(a):
    return np.ascontiguousarray(a.astype(ml_dtypes.bfloat16))


def _pairize_cols(W, hg, scale=1.0):
    """[768, 768] -> [768, 512]: pair p gets head hg*8+2p at cols 0:48 and
    head hg*8+2p+1 at cols 64:112 of its 128-col block; the rest zeros."""
    Wp = np.zeros((D, 512), np.float32)
    for p in range(NPAIR):
        ha = (hg * 8 + 2 * p) * DH
        hb = (hg * 8 + 2 * p + 1) * DH
        Wp[:, p * P:p * P + DH] = W[:, ha:ha + DH]
        Wp[:, p * P + 64:p * P + 64 + DH] = W[:, hb:hb + DH]
    if scale != 1.0:
        Wp *= scale
    return Wp


def _pairize_bias(b, hg, scale=1.0):
    bp = np.zeros((P, NPAIR), np.float32)
    for p in range(NPAIR):
        ha = (hg * 8 + 2 * p) * DH
        hb = (hg * 8 + 2 * p + 1) * DH
        bp[0:DH, p] = b[ha:ha + DH]
        bp[64:64 + DH, p] = b[hb:hb + DH]
    if scale != 1.0:
        bp *= scale
    return bp


def _prep_inputs(inputs):
    x = np.asarray(inputs["x"], np.float32)
    WQ = np.asarray(inputs["WQ"], np.float32)
    WK = np.asarray(inputs["WK"], np.float32)
    WV = np.asarray(inputs["WV"], np.float32)
    WP = np.asarray(inputs["WP"], np.float32)
    bQ = np.asarray(inputs["bQ"], np.float32)
    bK = np.asarray(inputs["bK"], np.float32)
    bV = np.asarray(inputs["bV"], np.float32)
    scale = 1.0 / math.sqrt(D)

    per_hg = {}
    for hg in range(2):
        wq_d = _bf16(_pairize_cols(WQ, hg, scale).reshape(CC, P, NPAIR, P).transpose(1, 2, 0, 3))
        wk_d = _bf16(_pairize_cols(WK, hg).reshape(CC, P, NPAIR, P).transpose(1, 2, 0, 3))
        wv_d = _bf16(WV[:, hg * 384:(hg + 1) * 384].reshape(CC, P, 384).transpose(1, 0, 2))
        WPpad = np.zeros((NPAIR, P, D), np.float32)
        for p in range(NPAIR):
            ha = (hg * 8 + 2 * p) * DH
            hb = (hg * 8 + 2 * p + 1) * DH
            WPpad[p, 0:DH] = WP[ha:ha + DH, :]
            WPpad[p, 64:64 + DH] = WP[hb:hb + DH, :]
        wp_d = _bf16(WPpad.reshape(NPAIR, P, CC, P).transpose(1, 0, 2, 3))
        bq_d = _pairize_bias(bQ, hg, scale)
        bk_d = _pairize_bias(bK, hg)
        bv_d = np.ascontiguousarray(
            np.broadcast_to(bV[hg * 384:(hg + 1) * 384].reshape(8, DH), (P, 8, DH))
        )
        per_hg[hg] = dict(wq=wq_d, wk=wk_d, wv=wv_d, wp=wp_d, bq=bq_d, bk=bk_d, bv=bv_d)

    in_maps = []
    for c in range(8):
        b, hg = c // 2, c % 2
        xt_d = _bf16(x[b].T.reshape(CC, P, N).transpose(1, 0, 2))
        m = dict(per_hg[hg])
        m["xt"] = xt_d
        in_maps.append(m)
    return in_maps


def kernel(**inputs):
    global LAST_RESULTS
    bP = np.asarray(inputs["bP"], np.float32)
    nc = _get_program()
    in_maps = _prep_inputs(inputs)
    trace = bool(os.environ.get("BASS_KERNEL_TRACE"))
    tmpdir = os.environ.get("BASS_KERNEL_TMPDIR") or None
    res = run_bass_kernel_spmd(nc, in_maps, list(range(8)), trace=trace, tmpdir=tmpdir)
    LAST_RESULTS = res
    out = np.empty((B, N, D), np.float32)
    for b in range(B):
        t = res.results[2 * b]["outt"].reshape(D, N) + \
            res.results[2 * b + 1]["outt"].reshape(D, N)
        out[b] = t.T + bP
    return out

